# revision 19
# baseline (speedup 1.0000x reference)
"""Trainium2 Bass kernel for nn_DifferentiableColorMLPRenderer.

Sharding: data-parallel over the batch - core b renders image b (B=8 images,
8 NeuronCores). The tiny MLP weights and (host-joined) per-face feature table
are replicated to every core.

Device pipeline per core (512x512 px; device pixel (p, B) = image flat pixel
p*2048 + B; B = block = one 128-px column of pf_s). Superblock = 16 blocks:

  1. one batched indirect-DMA gather per superblock: 2048 rows of 18B
     (bf16 per-face 9-vector G[pf], G = feature[faces] joined on host)
     land in 32-col stanzas: text[p, 32s+(3v+c)] = G[pf[p,16*sup+s]][3v+c]
  2. in-place DVE/GPSIMD mult by bary (free-dim broadcast); stanza col 9
     holds a persistent 1.0 (bias row trick)
  3. 4x PE transpose (bf16) -> psum1 [128,512]: row 32j+k = stanza val k of
     block 4g+j, col 128g+p
  4. DVE tensor_copy psum1 -> sb1 (bf16, 2x mode)
  5. L1: 4 row-tiled matmuls (tile_position (32j,0)), lhsT = per-strip
     [W1/3 tiled; b1; 0...] bf16, rhs = sb1[32j:32j+32, :], N=512
     -> p1 [128, 2048] (bias via the const-1 stanza row)
  6. relu1 on ACT -> h1 fp32r
  7. L2: 4x matmul fp32r N=512 -> p2; relu2 = tensor_scalar(add b2, max 0)
     on DVE -> h2 fp32r
  8. L3: 4x col-tiled matmul (tile_position (0,32j)), lhsT = W3 in cols
     4q+c (q = sup%4) of an M=16 strip, zero elsewhere; 4 superblocks
     ACCUMULATE into psum3[32j:32j+16, :] (phase q adds its channels at
     partitions 32j+4q+c, zeros elsewhere)
  9. relu3 + b3 bias on ACT every 4 sups -> sb3 [128, 1024]
 10. per 8 superblocks (2 relu3 outputs): one [128,1024] HWDGE DMA
     (~8.4MB/core output, host extracts the 16 used partitions/group)

Mask channel, background pixels (pf<=0, ~1e-5 of pixels), and the final
"-1.0" offset are applied on the host (they are pure functions of
pix_to_face plus an affine output fixup).
"""

import numpy as np
import jax
import ml_dtypes
import concourse.bacc as bacc
import concourse.bass as bass
import concourse.mybir as mybir
from concourse.tile import TileContext
from concourse import bass2jax
from concourse.bass2jax import _bass_exec_p, install_neuronx_cc_hook, partition_id_tensor
from jax.sharding import Mesh, NamedSharding, PartitionSpec
from jax.experimental.shard_map import shard_map

B, H, W = 8, 512, 512
V, F = 50000, 100000
P = 128
COLS = (H * W) // P          # 2048 blocks of 128 pixels per core
SUP = 16                     # blocks per superblock
MEGA = 8                     # superblocks per output DMA batch
NPIX = P * COLS

BF16 = ml_dtypes.bfloat16

_CACHE = {}


def _build_kernel(cols=COLS, n_cores=B, tmult_gpsimd=True,
                  relu1_dve=True, relu2_acts=4, dbg=None):
    nsup = cols // SUP
    assert cols % (SUP * MEGA) == 0
    nmega = nsup // MEGA

    nc = bacc.Bacc("TRN2", target_bir_lowering=False, debug=False,
                   num_devices=n_cores)
    dt = mybir.dt
    pf = nc.dram_tensor("pf", [P, cols], dt.int32, kind="ExternalInput")
    bary = nc.dram_tensor("bary", [P, cols * 3], dt.bfloat16,
                          kind="ExternalInput")
    gtab = nc.dram_tensor("gtab", [F, 9], dt.float32, kind="ExternalInput")
    w1x4 = nc.dram_tensor("w1x4", [P, 128], dt.bfloat16, kind="ExternalInput")
    identb = nc.dram_tensor("identb", [P, 128], dt.bfloat16,
                            kind="ExternalInput")
    w2 = nc.dram_tensor("w2", [128, 128], dt.float32, kind="ExternalInput")
    b2c = nc.dram_tensor("b2c", [128, 1], dt.float32, kind="ExternalInput")
    w3x16 = nc.dram_tensor("w3x16", [128, 2048], dt.float32,
                           kind="ExternalInput")
    b3x4 = nc.dram_tensor("b3x4", [128, 1], dt.float32, kind="ExternalInput")
    out = nc.dram_tensor("out", [128, (nsup // 4) * 512], dt.float32,
                         kind="ExternalOutput")
    dbg_out = None
    if dbg is not None:
        dbg_shape = {"text": [P, 512], "sb1": [P, 512], "h1": [P, 2048],
                     "h2": [P, 2048]}[dbg]
        dbg_dt = dt.bfloat16 if dbg in ("text", "sb1") else dt.float32
        dbg_out = nc.dram_tensor("dbg", dbg_shape, dbg_dt,
                                 kind="ExternalOutput")

    mm = mybir.AluOpType

    with TileContext(nc) as tc:
        with (
            tc.tile_pool(name="pp1", bufs=1, space="PSUM") as pp1,
            tc.tile_pool(name="pp2", bufs=1, space="PSUM") as pp2,
            tc.tile_pool(name="pp3", bufs=1, space="PSUM") as pp3,
            tc.tile_pool(name="ppt", bufs=1, space="PSUM") as ppt,
            tc.tile_pool(name="const", bufs=1) as cpool,
            tc.tile_pool(name="sbuf", bufs=2) as pool,
        ):
            # ---- constants ----
            ident_s = cpool.tile([P, 128], dt.bfloat16, tag="ident")
            nc.sync.dma_start(out=ident_s[:], in_=identb[:])
            w1x4_s = cpool.tile([P, 128], dt.bfloat16, tag="w1x4")
            nc.sync.dma_start(out=w1x4_s[:], in_=w1x4[:])
            w2_s = cpool.tile([128, 128], dt.float32, tag="w2")
            nc.sync.dma_start(out=w2_s[:], in_=w2[:])
            w2r_s = cpool.tile([128, 128], dt.float32r, tag="w2r")
            nc.vector.tensor_copy(out=w2r_s[:], in_=w2_s[:])
            w3x16_s = cpool.tile([128, 2048], dt.float32, tag="w3x16")
            nc.sync.dma_start(out=w3x16_s[:], in_=w3x16[:])
            w3x16r_s = cpool.tile([128, 2048], dt.float32r, tag="w3x16r")
            nc.vector.tensor_copy(out=w3x16r_s[:], in_=w3x16_s[:])
            b2_s = cpool.tile([128, 1], dt.float32, tag="b2")
            nc.sync.dma_start(out=b2_s[:], in_=b2c[:])
            b3x4_s = cpool.tile([128, 1], dt.float32, tag="b3x4")
            nc.sync.dma_start(out=b3x4_s[:], in_=b3x4[:])

            pf_s = cpool.tile([P, cols], dt.int32, tag="pf")
            nc.sync.dma_start(out=pf_s[:], in_=pf[:])
            bary_s = cpool.tile([P, cols * 3], dt.bfloat16, tag="bary")
            nc.sync.dma_start(out=bary_s[:], in_=bary[:])

            # rotating text buffers; col 32s+9 stays 1.0 forever (L1 bias
            # row), cols 32s+{10..31} stay 0 (zero weight rows)
            n_text = 3
            texts = []
            for i in range(n_text):
                tx = cpool.tile([P, 512], dt.bfloat16, tag=f"text{i}")
                nc.vector.memset(tx[:], 0)
                txa = tx[:]
                ones_view = bass.AP(txa.tensor, txa.offset + 9,
                                    [txa.ap[0], [32, SUP]])
                nc.vector.memset(ones_view, 1.0)
                texts.append(tx)

            # persistent psum tiles (15KB/partition total):
            # p1 [128,1024] x2 (8KB) + p2 [128,512] x2 (4KB, in loop)
            # + psum3 (2KB) + psum1 bf16 (1KB)
            psum3 = pp3.tile([128, 512], dt.float32, tag="psum3")
            psum1d = ppt.tile([128, 1024], dt.bfloat16, tag="psum1d")
            nc.vector.memset(psum3[:], 0)

            teng = nc.gpsimd if tmult_gpsimd else nc.vector
            mmop = mm

            sb1_of = {}
            sb3_box = [None]

            def fetch(s):
                text = texts[s % n_text]
                txa = text[:]
                # HW indirect DMA semantics: ONE index per partition per call
                for k in range(SUP):
                    nc.gpsimd.indirect_dma_start(
                        out=text[:, 32 * k:32 * k + 9], out_offset=None,
                        in_=gtab[:],
                        in_offset=bass.IndirectOffsetOnAxis(
                            ap=pf_s[:, SUP * s + k:SUP * s + k + 1], axis=0),
                    )
                tv = bass.AP(txa.tensor, txa.offset,
                             [txa.ap[0], [32, SUP], [3, 3], [1, 3]])
                ba = bary_s[:]
                bv = bass.AP(ba.tensor, ba.offset + 3 * SUP * s,
                             [ba.ap[0], [3, SUP], [1, 3], [0, 3]])
                teng.tensor_tensor(out=tv, in0=tv, in1=bv, op=mmop.mult)
                if dbg == "text" and s == 0:
                    nc.sync.dma_start(out=dbg_out[:], in_=text[:])

            def mid(s):
                text = texts[s % n_text]
                psum1 = psum1d[:, 512 * (s % 2):512 * (s % 2 + 1)]
                for g in range(4):
                    nc.tensor.transpose(out=psum1[:, 128 * g:128 * (g + 1)],
                                        in_=text[:, 128 * g:128 * (g + 1)],
                                        identity=ident_s[:])
                sb1 = pool.tile([128, 512], dt.bfloat16, tag="sb1", bufs=2)
                nc.vector.tensor_copy(out=sb1[:], in_=psum1)
                if dbg == "sb1" and s == 0:
                    nc.sync.dma_start(out=dbg_out[:], in_=sb1[:])
                sb1_of[s] = sb1

            def back(s):
                sb1 = sb1_of.pop(s)
                h1 = pool.tile([128, 2048], dt.float32r, tag="h1", bufs=2)
                for half in range(2):
                    p1 = pp1.tile([128, 1024], dt.float32, tag="p1", bufs=2)
                    for jj in range(2):
                        j = 2 * half + jj
                        nc.tensor.matmul(out=p1[:, 512 * jj:512 * (jj + 1)],
                                         lhsT=w1x4_s[32 * j:32 * (j + 1), :],
                                         rhs=sb1[32 * j:32 * (j + 1), :],
                                         start=True, stop=True,
                                         tile_position=(32 * j, 0))
                    if relu1_dve:
                        nc.vector.tensor_scalar(
                            out=h1[:, 1024 * half:1024 * (half + 1)],
                            in0=p1[:], scalar1=0.0, scalar2=None,
                            op0=mmop.max)
                    else:
                        nc.scalar.activation(
                            h1[:, 1024 * half:1024 * (half + 1)], p1[:],
                            mybir.ActivationFunctionType.Relu)
                if dbg == "h1" and s == 0:
                    nc.sync.dma_start(out=dbg_out[:],
                                      in_=h1[:].bitcast(dt.float32))
                h2 = pool.tile([128, 2048], dt.float32r, tag="h2", bufs=2)
                for j in range(4):
                    p2 = pp2.tile([128, 512], dt.float32, tag="p2", bufs=2)
                    nc.tensor.matmul(out=p2[:], lhsT=w2r_s[:],
                                     rhs=h1[:, 512 * j:512 * (j + 1)],
                                     start=True, stop=True)
                    if j < relu2_acts:
                        nc.scalar.activation(
                            h2[:, 512 * j:512 * (j + 1)], p2[:],
                            mybir.ActivationFunctionType.Relu, bias=b2_s[:])
                    else:
                        nc.vector.tensor_scalar(
                            out=h2[:, 512 * j:512 * (j + 1)], in0=p2[:],
                            scalar1=b2_s[:], scalar2=0.0,
                            op0=mmop.add, op1=mmop.max)
                if dbg == "h2" and s == 0:
                    nc.sync.dma_start(out=dbg_out[:],
                                      in_=h2[:].bitcast(dt.float32))
                q = s % 4
                for j in range(4):
                    v = 4 * q + j
                    nc.tensor.matmul(out=psum3[:, :],
                                     lhsT=w3x16r_s[:, 128 * v:128 * (v + 1)],
                                     rhs=h2[:, 512 * j:512 * (j + 1)],
                                     start=(q == 0 and j == 0),
                                     stop=(q == 3 and j == 3),
                                     skip_group_check=True)
                if q == 3:
                    t4 = s // 4
                    if t4 % 2 == 0:
                        sb3_new = pool.tile([128, 1024], dt.float32,
                                            tag="sb3", bufs=2)
                        sb3_box[0] = sb3_new
                    sb3 = sb3_box[0]
                    nc.scalar.activation(
                        sb3[:, 512 * (t4 % 2):512 * (t4 % 2 + 1)],
                        psum3[:], mybir.ActivationFunctionType.Relu,
                        bias=b3x4_s[:])
                    if t4 % 2 == 1:
                        mega = t4 // 2
                        nc.sync.dma_start(
                            out=out[:, mega * 1024:(mega + 1) * 1024],
                            in_=sb3[:])

            for it in range(nsup + 2):
                if it < nsup:
                    fetch(it)
                if 1 <= it <= nsup:
                    mid(it - 1)
                if it >= 2:
                    back(it - 2)
    nc.compile()
    return nc


def _make_callable(nc, n_cores):
    install_neuronx_cc_hook()
    partition_name = nc.partition_id_tensor.name if nc.partition_id_tensor else None
    in_names, out_names, out_avals, zero_outs = [], [], [], []
    for alloc in nc.m.functions[0].allocations:
        if not isinstance(alloc, mybir.MemoryLocationSet):
            continue
        name = alloc.memorylocations[0].name
        if alloc.kind == "ExternalInput":
            if name != partition_name:
                in_names.append(name)
        elif alloc.kind == "ExternalOutput":
            out_names.append(name)
            shape = tuple(alloc.tensor_shape)
            dtype = mybir.dt.np(alloc.dtype)
            out_avals.append(jax.core.ShapedArray(shape, dtype))
            zero_outs.append(np.zeros(shape, dtype))
    n_params = len(in_names)
    all_in_names = list(in_names) + list(out_names)
    if partition_name is not None:
        all_in_names.append(partition_name)

    def _body(*args):
        operands = list(args)
        if partition_name is not None:
            operands.append(partition_id_tensor())
        outs = _bass_exec_p.bind(
            *operands,
            out_avals=tuple(out_avals),
            in_names=tuple(all_in_names),
            out_names=tuple(out_names),
            lowering_input_output_aliases=(),
            sim_require_finite=True,
            sim_require_nnan=True,
            nc=nc,
        )
        return tuple(outs)

    devices = jax.devices()[:n_cores]
    mesh = Mesh(np.asarray(devices), ("core",))
    in_specs = tuple(
        PartitionSpec() if n in REPLICATED else PartitionSpec("core")
        for n in in_names) + (PartitionSpec("core"),) * len(out_names)
    out_specs = (PartitionSpec("core"),) * len(out_names)
    fn = jax.jit(
        shard_map(_body, mesh=mesh, in_specs=in_specs, out_specs=out_specs,
                  check_rep=False),
        keep_unused=True,
    )
    return fn, in_names, out_names, zero_outs, mesh


REPLICATED = {"gtab", "w1x4", "identb", "w2", "b2c", "w3x16", "b3x4"}


def _prep_in_maps(pix_to_face, bary_coords, faces, feature,
                  W1, b1, W2, b2, W3, b3):
    # host-side O(F) table join: per-face 9-vector of its 3 vertices'
    # features, bf16 (halves the random-gather traffic)
    gtab = np.ascontiguousarray(
        feature.astype(np.float32)[faces.astype(np.int64)].reshape(F, 9))
    strip = np.zeros((32, 128), np.float32)
    strip[0:9] = np.tile(W1.astype(np.float32) / 3.0, (3, 1))
    strip[9] = b1.astype(np.float32)
    w1x4 = np.tile(strip, (4, 1)).astype(BF16)
    # variant v=4q+j (cols 128v..128v+127): col 32j+4q+c = W3[:, c], else 0
    w3x16 = np.zeros((128, 2048), np.float32)
    for q in range(4):
        for j in range(4):
            v = 4 * q + j
            w3x16[:, 128 * v + 32 * j + 4 * q:
                  128 * v + 32 * j + 4 * q + 3] = W3.astype(np.float32)
    b3pat = np.zeros(32, np.float32)
    for q in range(4):
        b3pat[4 * q:4 * q + 3] = b3.astype(np.float32)
    b3x4 = np.tile(b3pat, 4).reshape(128, 1)
    shared = {
        "gtab": gtab,
        "w1x4": np.ascontiguousarray(w1x4),
        "identb": np.eye(128, dtype=BF16),
        "w2": W2.astype(np.float32),
        "b2c": b2.reshape(128, 1).astype(np.float32),
        "w3x16": np.ascontiguousarray(w3x16),
        "b3x4": np.ascontiguousarray(b3x4),
    }
    in_maps = []
    for b in range(B):
        m = dict(shared)
        m["pf"] = np.ascontiguousarray(
            pix_to_face[b, :, :, 0].reshape(P, COLS).astype(np.int32))
        m["bary"] = np.ascontiguousarray(
            bary_coords[b, :, :, 0, :].reshape(P, COLS * 3)).astype(BF16)
        in_maps.append(m)
    return in_maps


def _get_runner():
    if "runner" not in _CACHE:
        nc = _build_kernel()
        fn, in_names, out_names, zero_outs, mesh = _make_callable(nc, B)
        _CACHE["runner"] = (fn, in_names, out_names, zero_outs, mesh)
    return _CACHE["runner"]


def prepare(in_maps):
    fn, in_names, out_names, zero_outs, mesh = _get_runner()
    sh_core = NamedSharding(mesh, PartitionSpec("core"))
    sh_rep = NamedSharding(mesh, PartitionSpec())
    args = []
    for name in in_names:
        if name in REPLICATED:
            args.append(jax.device_put(np.asarray(in_maps[0][name]), sh_rep))
        else:
            a = np.concatenate([np.asarray(m[name]) for m in in_maps], axis=0)
            args.append(jax.device_put(a, sh_core))
    if "zeros" not in _CACHE:
        _CACHE["zeros"] = [
            jax.device_put(
                np.zeros((B * z.shape[0], *z.shape[1:]), z.dtype), sh_core)
            for z in zero_outs]
    args.extend(_CACHE["zeros"])
    return (fn, out_names, args)


def execute(handle):
    fn, out_names, args = handle
    outs = fn(*args)
    jax.block_until_ready(outs)
    res = []
    for c in range(B):
        d = {}
        for i, name in enumerate(out_names):
            a = np.asarray(outs[i])
            per = a.shape[0] // B
            d[name] = a[c * per:(c + 1) * per]
        res.append(d)
    return res


def run_on_device(in_maps):
    return execute(prepare(in_maps))


def decode_out(arr, cols=COLS):
    """[128, (nsup/4)*512] device buffer -> [H*W, 3] relu3 colors (device
    pixel (p, B) = image flat p*cols + B). Partition 32j+4q+c; col
    t*512 + 128g + p; block B = (4t+q)*16 + 4g + j."""
    nsup = cols // SUP
    A = arr.reshape(4, 8, 4, nsup // 4, 4, 128)[:, :4]  # [j,q,c,t,g,p]
    img = A.transpose(5, 3, 1, 4, 0, 2).reshape(P * cols, 4)
    return img[:, :3]


def kernel(pix_to_face, bary_coords, faces, feature,
           W1, b1, W2, b2, W3, b3):
    pix_to_face = np.asarray(pix_to_face)
    bary_coords = np.asarray(bary_coords)
    faces = np.asarray(faces)
    feature = np.asarray(feature)
    in_maps = _prep_in_maps(pix_to_face, bary_coords, faces, feature,
                            np.asarray(W1), np.asarray(b1), np.asarray(W2),
                            np.asarray(b2), np.asarray(W3), np.asarray(b3))
    results = run_on_device(in_maps)
    out = np.empty((B, H * W, 4), np.float32)
    pf_flat = pix_to_face[..., 0].reshape(B, H * W)
    for b in range(B):
        colors = decode_out(results[b]["out"])
        mask = pf_flat[b] > 0
        out[b, :, :3] = np.where(mask[:, None], colors - 1.0, 1.0)
        out[b, :, 3] = mask.astype(np.float32)
    return out.reshape(B, H, W, 4)
